# revision 1
# baseline (speedup 1.0000x reference)
"""Trainium2 Bass kernel for GNN NodeBlock (segment-sum + MLP + LayerNorm + residual).

Strategy: shard NODES across the 8 cores (no collectives needed).
Host-side, nodes are packed into 392 windows of 128 nodes with balanced edge
counts (LPT), and every edge is routed to the window that owns its destination
node. Each window's edges are padded to CH chunks of 128. On device, each
128-edge chunk is segment-summed into its window via a one-hot matmul
accumulated in PSUM ([feat, node] orientation), then the MeshGraphMLP
(Linear->SiLU->Linear->LayerNorm) + residual runs per window entirely on-chip.
"""
import os
os.environ.setdefault("JAX_PLATFORMS", "axon,cpu")
import sys
if "/opt/trn_rl_repo" not in sys.path:
    sys.path.insert(0, "/opt/trn_rl_repo")

import numpy as np

N_NODES = 50000
N_EDGES_REF = 800000
D = 128
HID = 128
P = 128                      # nodes per window / partition count
N_CORES = 8
W = 49                       # windows per core
W_TOT = N_CORES * W          # 392
NODE_SLOTS = W_TOT * P       # 50176
BATCH = 7                    # windows per output DMA (49 = 7*7)

_program_cache: dict = {}


# ----------------------------------------------------------------------------
# Host-side preprocessing
# ----------------------------------------------------------------------------

def _balance_windows(deg):
    """LPT-assign nodes to W_TOT windows of <=P nodes, balancing edge sums."""
    import heapq
    order = np.argsort(-deg, kind="stable")
    heap = [(0, w) for w in range(W_TOT)]
    heapq.heapify(heap)
    slots_used = np.zeros(W_TOT, np.int32)
    win_sum = np.zeros(W_TOT, np.int64)
    node_win = np.empty(len(deg), np.int32)
    node_pos = np.empty(len(deg), np.int32)
    for n in order:
        while True:
            s, w = heapq.heappop(heap)
            if slots_used[w] < P:
                break
        node_win[n] = w
        node_pos[n] = slots_used[w]
        slots_used[w] += 1
        win_sum[w] += deg[n]
        if slots_used[w] < P:
            heapq.heappush(heap, (int(win_sum[w]), w))
    return node_win, node_pos, win_sum


def _preprocess(efeat, nfeat, dst_idx, ln_b):
    n_nodes = nfeat.shape[0]
    n_edges = efeat.shape[0]
    dst = np.asarray(dst_idx).astype(np.int64)
    deg = np.bincount(dst, minlength=n_nodes)
    node_win, node_pos, win_sum = _balance_windows(deg)
    e_max = int(win_sum.max())
    CH = max((e_max + P - 1) // P, 1)

    # Route each edge to (window, chunk, partition) of its destination node.
    win_of_edge = node_win[dst]
    edge_perm = np.argsort(win_of_edge, kind="stable")
    wsorted = win_of_edge[edge_perm]
    counts = np.bincount(wsorted, minlength=W_TOT)
    starts = np.concatenate([[0], np.cumsum(counts)[:-1]])
    j_within = np.arange(n_edges, dtype=np.int64) - np.repeat(starts, counts)
    c = j_within // P
    p = j_within % P
    flat_row = (wsorted.astype(np.int64) * P + p) * CH + c

    efeat_dev = np.zeros((W_TOT * P * CH, D), np.float32)
    efeat_dev[flat_row] = efeat[edge_perm]
    rel_dev = np.zeros((W_TOT * P, CH), np.float32)
    rel_dev[wsorted.astype(np.int64) * P + p, c] = node_pos[dst[edge_perm]].astype(np.float32)

    nfeat_perm = np.zeros((NODE_SLOTS, D), np.float32)
    slot_of_node = node_win.astype(np.int64) * P + node_pos
    nfeat_perm[slot_of_node] = nfeat
    nfb_perm = nfeat_perm + np.asarray(ln_b, np.float32)[None, :]

    return dict(efeat_dev=efeat_dev, rel_dev=rel_dev, nfeat_perm=nfeat_perm,
                nfb_perm=nfb_perm, slot_of_node=slot_of_node, CH=CH)


def _build_in_maps(pre, w1, b1, w2, b2, ln_g):
    CH = pre["CH"]
    efeat_dev = pre["efeat_dev"].reshape(W_TOT, P, CH, D)
    rel_dev = pre["rel_dev"].reshape(W_TOT, P, CH)
    nfeat_perm = pre["nfeat_perm"]
    nfb_perm = pre["nfb_perm"]

    iota = np.ascontiguousarray(
        np.broadcast_to(np.arange(P, dtype=np.float32), (P, CH, P)))
    w1 = np.asarray(w1, np.float32)
    w1a = np.ascontiguousarray(w1[:D])
    w1b = np.ascontiguousarray(w1[D:])
    w2c = np.ascontiguousarray(np.asarray(w2, np.float32))
    b1c = np.ascontiguousarray(np.asarray(b1, np.float32)[:, None])
    grep = np.ascontiguousarray(
        np.broadcast_to(np.asarray(ln_g, np.float32), (P, D)))
    b2rep = np.ascontiguousarray(
        np.broadcast_to(np.asarray(b2, np.float32), (P, D)))

    in_maps = []
    for cidx in range(N_CORES):
        sl = slice(cidx * W, (cidx + 1) * W)
        nsl = slice(cidx * W * P, (cidx + 1) * W * P)
        in_maps.append(dict(
            ef=np.ascontiguousarray(efeat_dev[sl]),
            rel=np.ascontiguousarray(rel_dev[sl].transpose(1, 0, 2)),
            iota=iota,
            nfT=np.ascontiguousarray(nfeat_perm[nsl].T),
            nfb=np.ascontiguousarray(
                nfb_perm[nsl].reshape(W, P, D).transpose(1, 0, 2)),
            w1a=w1a, w1b=w1b, w2=w2c, b1=b1c, grep=grep, b2rep=b2rep,
        ))
    return in_maps


# ----------------------------------------------------------------------------
# Device program
# ----------------------------------------------------------------------------

def _build_program(CH):
    import concourse.bass as bass
    import concourse.tile as tile
    from concourse import bacc, mybir
    from contextlib import ExitStack

    f32 = mybir.dt.float32
    nc = bacc.Bacc("TRN2", target_bir_lowering=False, debug=False,
                   enable_asserts=True, num_devices=N_CORES)

    ef = nc.dram_tensor("ef", [W, P, CH, D], f32, kind="ExternalInput").ap()
    rel = nc.dram_tensor("rel", [P, W, CH], f32, kind="ExternalInput").ap()
    iota = nc.dram_tensor("iota", [P, CH, P], f32, kind="ExternalInput").ap()
    nfT = nc.dram_tensor("nfT", [P, W * P], f32, kind="ExternalInput").ap()
    nfb = nc.dram_tensor("nfb", [P, W, D], f32, kind="ExternalInput").ap()
    w1a = nc.dram_tensor("w1a", [D, HID], f32, kind="ExternalInput").ap()
    w1b = nc.dram_tensor("w1b", [D, HID], f32, kind="ExternalInput").ap()
    w2 = nc.dram_tensor("w2", [HID, D], f32, kind="ExternalInput").ap()
    b1 = nc.dram_tensor("b1", [HID, 1], f32, kind="ExternalInput").ap()
    grep = nc.dram_tensor("grep", [P, D], f32, kind="ExternalInput").ap()
    b2rep = nc.dram_tensor("b2rep", [P, D], f32, kind="ExternalInput").ap()
    out = nc.dram_tensor("out", [P, W * D], f32, kind="ExternalOutput").ap()

    with ExitStack() as ctx:
        tc = ctx.enter_context(tile.TileContext(nc))
        consts = ctx.enter_context(tc.tile_pool(name="consts", bufs=1))
        ef_pool = ctx.enter_context(tc.tile_pool(name="ef", bufs=3))
        oh_pool = ctx.enter_context(tc.tile_pool(name="oh", bufs=2))
        agg_pool = ctx.enter_context(tc.tile_pool(name="agg", bufs=2))
        h_pool = ctx.enter_context(tc.tile_pool(name="h", bufs=2))
        x_pool = ctx.enter_context(tc.tile_pool(name="x", bufs=4))
        out_pool = ctx.enter_context(tc.tile_pool(name="outp", bufs=2))
        stat_pool = ctx.enter_context(tc.tile_pool(name="stat", bufs=8))
        agg_ps = ctx.enter_context(tc.tile_pool(name="agg_ps", bufs=2, space="PSUM"))
        h1_ps = ctx.enter_context(tc.tile_pool(name="h1_ps", bufs=2, space="PSUM"))
        o2_ps = ctx.enter_context(tc.tile_pool(name="o2_ps", bufs=2, space="PSUM"))

        # Load constants (SWDGE so the big HWDGE queues stay free)
        t_iota = consts.tile([P, CH, P], f32)
        nc.gpsimd.dma_start(out=t_iota[:], in_=iota[:])
        t_rel = consts.tile([P, W, CH], f32)
        nc.gpsimd.dma_start(out=t_rel[:], in_=rel[:])
        t_nfT = consts.tile([P, W * P], f32)
        nc.gpsimd.dma_start(out=t_nfT[:], in_=nfT[:])
        t_nfb = consts.tile([P, W, D], f32)
        nc.gpsimd.dma_start(out=t_nfb[:], in_=nfb[:])
        t_w1a = consts.tile([D, HID], f32)
        nc.gpsimd.dma_start(out=t_w1a[:], in_=w1a[:])
        t_w1b = consts.tile([D, HID], f32)
        nc.gpsimd.dma_start(out=t_w1b[:], in_=w1b[:])
        t_w2 = consts.tile([HID, D], f32)
        nc.gpsimd.dma_start(out=t_w2[:], in_=w2[:])
        t_b1 = consts.tile([HID, 1], f32)
        nc.gpsimd.dma_start(out=t_b1[:], in_=b1[:])
        t_grep = consts.tile([P, D], f32)
        nc.gpsimd.dma_start(out=t_grep[:], in_=grep[:])
        t_b2rep = consts.tile([P, D], f32)
        nc.gpsimd.dma_start(out=t_b2rep[:], in_=b2rep[:])
        t_eps = consts.tile([P, 1], f32)
        nc.vector.memset(t_eps[:], 1e-5)

        AF = mybir.ActivationFunctionType
        OP = mybir.AluOpType

        out_tile = None
        for w in range(W):
            b = w % BATCH
            if b == 0:
                out_tile = out_pool.tile([P, BATCH * D], f32)

            eft = ef_pool.tile([P, CH, D], f32)
            nc.sync.dma_start(out=eft[:], in_=ef[w])

            # one-hot selection: oh[p, c, v] = (rel[p, w, c] == v)
            oh = oh_pool.tile([P, CH, P], f32)
            nc.vector.tensor_tensor(
                out=oh[:],
                in0=t_rel[:, w, :, None].to_broadcast([P, CH, P]),
                in1=t_iota[:],
                op=OP.is_equal,
            )

            # aggT[f, v] = sum_c eft[:, c, :].T @ oh[:, c, :]
            aggp = agg_ps.tile([P, P], f32, space="PSUM")
            for c in range(CH):
                nc.tensor.matmul(
                    out=aggp[:],
                    lhsT=eft[:, c, :],
                    rhs=oh[:, c, :],
                    start=(c == 0),
                    stop=(c == CH - 1),
                )
            aggs = agg_pool.tile([P, P], f32)
            nc.vector.tensor_copy(out=aggs[:], in_=aggp[:])

            # h1T[hid, v] = w1a.T @ aggT + w1b.T @ nfT_w ; h = silu(h1T + b1)
            h1p = h1_ps.tile([HID, P], f32, space="PSUM")
            nc.tensor.matmul(out=h1p[:], lhsT=t_w1a[:], rhs=aggs[:],
                             start=True, stop=False)
            nc.tensor.matmul(out=h1p[:], lhsT=t_w1b[:],
                             rhs=t_nfT[:, w * P:(w + 1) * P],
                             start=False, stop=True)
            h = h_pool.tile([HID, P], f32)
            nc.scalar.activation(out=h[:], in_=h1p[:], func=AF.Silu,
                                 bias=t_b1[:], scale=1.0)

            # o2[v, f] = h.T @ w2 ; x = o2 + b2
            o2p = o2_ps.tile([P, D], f32, space="PSUM")
            nc.tensor.matmul(out=o2p[:], lhsT=h[:], rhs=t_w2[:],
                             start=True, stop=True)
            x = x_pool.tile([P, D], f32)
            nc.vector.tensor_tensor(out=x[:], in0=o2p[:], in1=t_b2rep[:],
                                    op=OP.add)

            # LayerNorm over features (free dim)
            stats = stat_pool.tile([P, 6], f32)
            nc.vector.bn_stats(out=stats[:], in_=x[:])
            mv = stat_pool.tile([P, 2], f32)
            nc.vector.bn_aggr(out=mv[:], in_=stats[:])
            sd = stat_pool.tile([P, 1], f32)
            nc.scalar.activation(out=sd[:], in_=mv[:, 1:2], func=AF.Sqrt,
                                 bias=t_eps[:], scale=1.0)
            rstd = stat_pool.tile([P, 1], f32)
            nc.vector.reciprocal(out=rstd[:], in_=sd[:])
            xn = x_pool.tile([P, D], f32)
            nc.vector.tensor_scalar(out=xn[:], in0=x[:], scalar1=mv[:, 0:1],
                                    scalar2=rstd[:], op0=OP.subtract,
                                    op1=OP.mult)

            # out = xn * ln_g + (nfeat + ln_b)   (on gpsimd: DVE is busiest)
            xg = x_pool.tile([P, D], f32)
            nc.gpsimd.tensor_mul(out=xg[:], in0=xn[:], in1=t_grep[:])
            nc.gpsimd.tensor_add(out=out_tile[:, b * D:(b + 1) * D],
                                 in0=xg[:], in1=t_nfb[:, w, :])

            if b == BATCH - 1:
                blk = w // BATCH
                nc.scalar.dma_start(
                    out=out[:, blk * BATCH * D:(blk + 1) * BATCH * D],
                    in_=out_tile[:])

    nc.finalize()
    return nc


def _get_program(CH):
    if CH not in _program_cache:
        _program_cache[CH] = _build_program(CH)
    return _program_cache[CH]


# ----------------------------------------------------------------------------
# Entry point
# ----------------------------------------------------------------------------

def kernel(efeat, nfeat, dst_idx, w1, b1, w2, b2, ln_g, ln_b):
    from concourse.bass_utils import run_bass_kernel_spmd

    efeat = np.asarray(efeat, np.float32)
    nfeat = np.asarray(nfeat, np.float32)
    pre = _preprocess(efeat, nfeat, dst_idx, ln_b)
    nc = _get_program(pre["CH"])
    in_maps = _build_in_maps(pre, w1, b1, w2, b2, ln_g)

    res = run_bass_kernel_spmd(nc, in_maps, list(range(N_CORES)))

    out_slots = np.empty((NODE_SLOTS, D), np.float32)
    for cidx in range(N_CORES):
        oc = res.results[cidx]["out"].reshape(P, W, D).transpose(1, 0, 2)
        out_slots[cidx * W * P:(cidx + 1) * W * P] = oc.reshape(W * P, D)
    return out_slots[pre["slot_of_node"]]


# revision 6
# speedup vs baseline: 331.2420x; 331.2420x over previous
"""Trainium2 Bass kernel for GNN NodeBlock (segment-sum + MLP + LayerNorm + residual).

Strategy: shard NODES across the 8 cores (no collectives needed).

Host side packs nodes into GROUPS of <=8 nodes whose total in-degree is <=128
(snake-deal over degree-sorted nodes + local repair). Every edge is routed to
its destination node's group; a group's edges (padded to 128) form one matmul
chunk. 16 groups = one WINDOW of 128 node slots; 50 windows per core.

Device side, per window: for each of the 16 chunks, a single [128e x 128f]^T @
[128e x 8v] one-hot matmul segment-sums the chunk's edges into its own
disjoint 8-column slice of the window's PSUM accumulator ([feat, node]
orientation, no accumulation needed). Then the MeshGraphMLP
(Linear->SiLU->Linear->LayerNorm) + residual runs per window on-chip, with
sqrt batched across windows to avoid ACT table thrash. Edge features and
one-hots travel in bf16 (exact 0/1 one-hots; fp32 PSUM accumulate); everything
downstream of the segment-sum is fp32.
"""
import os
import sys
if "/opt/trn_rl_repo" not in sys.path:
    sys.path.insert(0, "/opt/trn_rl_repo")

import numpy as np

N_NODES = 50000
D = 128
HID = 128
P = 128                      # SBUF partitions / edges per chunk / nodes per window
N_CORES = 8
CH = 16                      # chunks (groups) per window
GN = 8                       # node slots per group
GE = 128                     # edge capacity per group
BATCH = 5                    # windows per output DMA / sqrt batch

_program_cache: dict = {}


# ----------------------------------------------------------------------------
# Host-side preprocessing
# ----------------------------------------------------------------------------

def _pack_groups(deg, n_groups):
    """Snake-deal degree-sorted nodes into groups of <=GN nodes / <=GE edges,
    then repair the few sum-cap violations by swapping with light groups.
    Returns (node_grp, node_rel) or None if infeasible."""
    n = len(deg)
    order = np.argsort(-deg, kind="stable")
    node_grp = np.full(n, -1, np.int32)
    for l in range(GN):
        lo, hi = l * n_groups, min((l + 1) * n_groups, n)
        if lo >= n:
            break
        idx = order[lo:hi]
        g = np.arange(hi - lo)
        if l % 2:
            g = n_groups - 1 - g
        node_grp[idx] = g
    gsum = np.bincount(node_grp, weights=deg, minlength=n_groups).astype(np.int64)
    members = [[] for _ in range(n_groups)]
    for node in order:
        members[node_grp[node]].append(node)

    over = list(np.where(gsum > GE)[0])
    if over:
        cand = np.argsort(gsum)[:4000].tolist()
        for g in over:
            guard = 0
            while gsum[g] > GE and guard < 200:
                guard += 1
                done = False
                for a in sorted(members[g], key=lambda x: -deg[x]):
                    for u in cand:
                        if u == g or gsum[u] > GE or not members[u]:
                            continue
                        b = min(members[u], key=lambda x: deg[x])
                        if deg[a] > deg[b] and gsum[u] - deg[b] + deg[a] <= GE:
                            members[g].remove(a)
                            members[u].remove(b)
                            members[g].append(b)
                            members[u].append(a)
                            node_grp[a], node_grp[b] = u, g
                            dd = int(deg[a] - deg[b])
                            gsum[g] -= dd
                            gsum[u] += dd
                            done = True
                            break
                    if done:
                        break
                if not done:
                    return None
    if gsum.max() > GE:
        return None
    node_rel = np.empty(n, np.int32)
    for g in range(n_groups):
        for i, node in enumerate(members[g]):
            node_rel[node] = i
    return node_grp, node_rel


def _preprocess(efeat, nfeat, dst_idx, ln_b):
    import ml_dtypes
    bf16 = np.dtype(ml_dtypes.bfloat16)
    n_nodes = nfeat.shape[0]
    n_edges = efeat.shape[0]
    dst = np.asarray(dst_idx).astype(np.int64)
    deg = np.bincount(dst, minlength=n_nodes)
    if deg.max() > GE:
        raise ValueError(f"node degree {deg.max()} exceeds group capacity {GE}")

    for W in (50, 51, 52, 54, 58, 64):
        n_groups = N_CORES * W * CH
        if n_groups * GN < n_nodes or n_groups * GE < n_edges:
            continue
        r = _pack_groups(deg, n_groups)
        if r is not None:
            break
    else:
        raise ValueError("group packing failed")
    node_grp, node_rel = r
    W_TOT = N_CORES * W
    node_slots = W_TOT * P

    # Route each edge to (window, chunk, partition) of its destination group.
    g_of_edge = node_grp[dst]
    edge_perm = np.argsort(g_of_edge, kind="stable")
    gsorted = g_of_edge[edge_perm]
    counts = np.bincount(gsorted, minlength=n_groups)
    starts = np.concatenate([[0], np.cumsum(counts)[:-1]])
    j_within = np.arange(n_edges, dtype=np.int64) - np.repeat(starts, counts)
    w = gsorted.astype(np.int64) // CH
    c = gsorted.astype(np.int64) % CH
    p = j_within
    flat_row = (w * P + p) * CH + c

    efeat_dev = np.zeros((W_TOT * P * CH, D), bf16)
    efeat_dev[flat_row] = efeat[edge_perm].astype(bf16)
    rel_dev = np.zeros((W_TOT * P, CH), bf16)
    rel_dev[w * P + p, c] = node_rel[dst[edge_perm]].astype(bf16)

    nfeat_perm = np.zeros((node_slots, D), np.float32)
    slot_of_node = node_grp.astype(np.int64) * GN + node_rel
    nfeat_perm[slot_of_node] = nfeat
    nfb_perm = nfeat_perm + np.asarray(ln_b, np.float32)[None, :]

    return dict(efeat_dev=efeat_dev, rel_dev=rel_dev, nfeat_perm=nfeat_perm,
                nfb_perm=nfb_perm, slot_of_node=slot_of_node, W=W)


def _build_in_maps(pre, w1, b1, w2, b2, ln_g):
    import ml_dtypes
    bf16 = np.dtype(ml_dtypes.bfloat16)
    W = pre["W"]
    W_TOT = N_CORES * W
    efeat_dev = pre["efeat_dev"].reshape(W_TOT, P, CH, D)
    rel_dev = pre["rel_dev"].reshape(W_TOT, P, CH)
    nfeat_perm = pre["nfeat_perm"]
    nfb_perm = pre["nfb_perm"]

    iota = np.ascontiguousarray(
        np.broadcast_to(np.arange(GN).astype(bf16), (P, CH, GN)))
    w1 = np.asarray(w1, np.float32)
    w1a = np.ascontiguousarray(w1[:D])
    w1b = np.ascontiguousarray(w1[D:])
    w2c = np.ascontiguousarray(np.asarray(w2, np.float32))
    b1c = np.ascontiguousarray(np.asarray(b1, np.float32)[:, None])
    grep = np.ascontiguousarray(
        np.broadcast_to(np.asarray(ln_g, np.float32), (P, D)))
    b2rep = np.ascontiguousarray(
        np.broadcast_to(np.asarray(b2, np.float32), (P, D)))

    in_maps = []
    for cidx in range(N_CORES):
        sl = slice(cidx * W, (cidx + 1) * W)
        nsl = slice(cidx * W * P, (cidx + 1) * W * P)
        in_maps.append(dict(
            ef=np.ascontiguousarray(efeat_dev[sl]),
            rel=np.ascontiguousarray(rel_dev[sl].transpose(1, 0, 2)),
            iota=iota,
            nfT=np.ascontiguousarray(nfeat_perm[nsl].T),
            nfb=np.ascontiguousarray(
                nfb_perm[nsl].reshape(W, P, D).transpose(1, 0, 2)),
            w1a=w1a, w1b=w1b, w2=w2c, b1=b1c, grep=grep, b2rep=b2rep,
        ))
    return in_maps


# ----------------------------------------------------------------------------
# Device program
# ----------------------------------------------------------------------------

def _build_program(W):
    import concourse.bass as bass
    import concourse.tile as tile
    from concourse import bacc, mybir
    from contextlib import ExitStack

    f32 = mybir.dt.float32
    bf16 = mybir.dt.bfloat16
    nc = bacc.Bacc("TRN2", target_bir_lowering=False, debug=False,
                   enable_asserts=True, num_devices=N_CORES)

    ef = nc.dram_tensor("ef", [W, P, CH, D], bf16, kind="ExternalInput").ap()
    rel = nc.dram_tensor("rel", [P, W, CH], bf16, kind="ExternalInput").ap()
    iota = nc.dram_tensor("iota", [P, CH, GN], bf16, kind="ExternalInput").ap()
    nfT = nc.dram_tensor("nfT", [P, W * P], f32, kind="ExternalInput").ap()
    nfb = nc.dram_tensor("nfb", [P, W, D], f32, kind="ExternalInput").ap()
    w1a = nc.dram_tensor("w1a", [D, HID], f32, kind="ExternalInput").ap()
    w1b = nc.dram_tensor("w1b", [D, HID], f32, kind="ExternalInput").ap()
    w2 = nc.dram_tensor("w2", [HID, D], f32, kind="ExternalInput").ap()
    b1 = nc.dram_tensor("b1", [HID, 1], f32, kind="ExternalInput").ap()
    grep = nc.dram_tensor("grep", [P, D], f32, kind="ExternalInput").ap()
    b2rep = nc.dram_tensor("b2rep", [P, D], f32, kind="ExternalInput").ap()
    out = nc.dram_tensor("out", [P, W * D], f32, kind="ExternalOutput").ap()

    with ExitStack() as ctx:
        tc = ctx.enter_context(tile.TileContext(nc))
        consts = ctx.enter_context(tc.tile_pool(name="consts", bufs=1))
        ef_pool = ctx.enter_context(tc.tile_pool(name="ef", bufs=3))
        oh_pool = ctx.enter_context(tc.tile_pool(name="oh", bufs=3))
        agg_pool = ctx.enter_context(tc.tile_pool(name="agg", bufs=2))
        h_pool = ctx.enter_context(tc.tile_pool(name="h", bufs=2))
        x_pool = ctx.enter_context(tc.tile_pool(name="x", bufs=2 * BATCH + 2))
        xn_pool = ctx.enter_context(tc.tile_pool(name="xn", bufs=3))
        out_pool = ctx.enter_context(tc.tile_pool(name="outp", bufs=2))
        mv_pool = ctx.enter_context(tc.tile_pool(name="mv", bufs=3))
        stat_pool = ctx.enter_context(tc.tile_pool(name="stat", bufs=6))
        agg_ps = ctx.enter_context(tc.tile_pool(name="agg_ps", bufs=2, space="PSUM"))
        h1_ps = ctx.enter_context(tc.tile_pool(name="h1_ps", bufs=2, space="PSUM"))
        o2_ps = ctx.enter_context(tc.tile_pool(name="o2_ps", bufs=2, space="PSUM"))

        # Load constants (SWDGE so the big HWDGE queues stay free)
        t_iota = consts.tile([P, CH, GN], bf16)
        nc.gpsimd.dma_start(out=t_iota[:], in_=iota[:])
        t_rel = consts.tile([P, W, CH], bf16)
        nc.gpsimd.dma_start(out=t_rel[:], in_=rel[:])
        t_nfT = consts.tile([P, W * P], f32)
        nc.gpsimd.dma_start(out=t_nfT[:], in_=nfT[:])
        t_nfb = consts.tile([P, W, D], f32)
        nc.gpsimd.dma_start(out=t_nfb[:], in_=nfb[:])
        t_w1a = consts.tile([D, HID], f32)
        nc.gpsimd.dma_start(out=t_w1a[:], in_=w1a[:])
        t_w1b = consts.tile([D, HID], f32)
        nc.gpsimd.dma_start(out=t_w1b[:], in_=w1b[:])
        t_w2 = consts.tile([HID, D], f32)
        nc.gpsimd.dma_start(out=t_w2[:], in_=w2[:])
        t_b1 = consts.tile([HID, 1], f32)
        nc.gpsimd.dma_start(out=t_b1[:], in_=b1[:])
        t_grep = consts.tile([P, D], f32)
        nc.gpsimd.dma_start(out=t_grep[:], in_=grep[:])
        t_b2rep = consts.tile([P, D], f32)
        nc.gpsimd.dma_start(out=t_b2rep[:], in_=b2rep[:])
        t_eps = consts.tile([P, 1], f32)
        nc.vector.memset(t_eps[:], 1e-5)

        AF = mybir.ActivationFunctionType
        OP = mybir.AluOpType

        xs = [None] * BATCH
        mv_b = None
        out_tile = None
        eft = None

        for w in range(W):
            b = w % BATCH
            blk = w // BATCH
            if b == 0:
                out_tile = out_pool.tile([P, BATCH * D], f32)
                mv_b = mv_pool.tile([P, BATCH, 2], f32, tag="mv")

            # efeat DMA: two windows (1MB bf16) at a time
            if w % 2 == 0:
                nw = min(2, W - w)
                eft = ef_pool.tile([P, 2, CH, D], bf16, tag="eft")
                nc.sync.dma_start(out=eft[:, :nw],
                                  in_=ef[w:w + nw].rearrange("w p c d -> p w c d"))
            efw = eft[:, w % 2]

            # one-hot: oh[p, c, v] = (rel[p, w, c] == v), v in [0, 8)
            oh = oh_pool.tile([P, CH, GN], bf16)
            nc.vector.tensor_tensor(
                out=oh[:],
                in0=t_rel[:, w, :, None].to_broadcast([P, CH, GN]),
                in1=t_iota[:],
                op=OP.is_equal,
            )

            # aggT[f, c*8+v] = efw[:, c, :].T @ oh[:, c, :]  (disjoint cols)
            aggp = agg_ps.tile([P, CH * GN], f32, space="PSUM")
            for c in range(CH):
                nc.tensor.matmul(
                    out=aggp[:, c * GN:(c + 1) * GN],
                    lhsT=efw[:, c, :],
                    rhs=oh[:, c, :],
                    start=True,
                    stop=True,
                )
            aggs = agg_pool.tile([P, P], f32)
            nc.any.tensor_copy(out=aggs[:], in_=aggp[:])

            # h1T[hid, v] = w1a.T @ aggT + w1b.T @ nfT_w ; h = silu(h1T + b1)
            h1p = h1_ps.tile([HID, P], f32, space="PSUM")
            nc.tensor.matmul(out=h1p[:], lhsT=t_w1a[:], rhs=aggs[:],
                             start=True, stop=False)
            nc.tensor.matmul(out=h1p[:], lhsT=t_w1b[:],
                             rhs=t_nfT[:, w * P:(w + 1) * P],
                             start=False, stop=True)
            h = h_pool.tile([HID, P], f32)
            nc.scalar.activation(out=h[:], in_=h1p[:], func=AF.Silu,
                                 bias=t_b1[:], scale=1.0)

            # o2[v, f] = h.T @ w2 ; x = o2 + b2
            o2p = o2_ps.tile([P, D], f32, space="PSUM")
            nc.tensor.matmul(out=o2p[:], lhsT=h[:], rhs=t_w2[:],
                             start=True, stop=True)
            x = x_pool.tile([P, D], f32, tag="x")
            nc.vector.tensor_tensor(out=x[:], in0=o2p[:], in1=t_b2rep[:],
                                    op=OP.add)

            # LayerNorm stats (normalize at batch end: one Sqrt per batch)
            stats = stat_pool.tile([P, 6], f32)
            nc.vector.bn_stats(out=stats[:], in_=x[:])
            nc.vector.bn_aggr(out=mv_b[:, b, :], in_=stats[:])
            xs[b] = x

            if b == BATCH - 1:
                sd_b = stat_pool.tile([P, BATCH], f32, tag="sd")
                nc.scalar.activation(out=sd_b[:], in_=mv_b[:, :, 1],
                                     func=AF.Sqrt, bias=t_eps[:], scale=1.0)
                rstd_b = stat_pool.tile([P, BATCH], f32, tag="rstd")
                nc.vector.reciprocal(out=rstd_b[:], in_=sd_b[:])

                for i in range(BATCH):
                    xn = xn_pool.tile([P, D], f32)
                    nc.vector.tensor_scalar(out=xn[:], in0=xs[i][:],
                                            scalar1=mv_b[:, i, 0:1],
                                            scalar2=rstd_b[:, i:i + 1],
                                            op0=OP.subtract, op1=OP.mult)
                    wg = blk * BATCH + i
                    xg = xn_pool.tile([P, D], f32, tag="xg")
                    nc.gpsimd.tensor_mul(out=xg[:], in0=xn[:], in1=t_grep[:])
                    nc.gpsimd.tensor_add(out=out_tile[:, i * D:(i + 1) * D],
                                         in0=xg[:], in1=t_nfb[:, wg, :])

                nc.scalar.dma_start(
                    out=out[:, blk * BATCH * D:(blk + 1) * BATCH * D],
                    in_=out_tile[:])

    nc.finalize()
    return nc


def _get_program(W):
    if W not in _program_cache:
        _program_cache[W] = _build_program(W)
    return _program_cache[W]


# ----------------------------------------------------------------------------
# Entry point
# ----------------------------------------------------------------------------

def kernel(efeat, nfeat, dst_idx, w1, b1, w2, b2, ln_g, ln_b):
    from concourse.bass_utils import run_bass_kernel_spmd

    efeat = np.asarray(efeat, np.float32)
    nfeat = np.asarray(nfeat, np.float32)
    pre = _preprocess(efeat, nfeat, dst_idx, ln_b)
    W = pre["W"]
    nc = _get_program(W)
    in_maps = _build_in_maps(pre, w1, b1, w2, b2, ln_g)

    res = run_bass_kernel_spmd(nc, in_maps, list(range(N_CORES)))

    node_slots = N_CORES * W * P
    out_slots = np.empty((node_slots, D), np.float32)
    for cidx in range(N_CORES):
        oc = res.results[cidx]["out"].reshape(P, W, D).transpose(1, 0, 2)
        out_slots[cidx * W * P:(cidx + 1) * W * P] = oc.reshape(W * P, D)
    return out_slots[pre["slot_of_node"]]


# revision 7
# speedup vs baseline: 334.4963x; 1.0098x over previous
"""Trainium2 Bass kernel for GNN NodeBlock (segment-sum + MLP + LayerNorm + residual).

Strategy: shard NODES across the 8 cores (no collectives needed).

Host side packs nodes into GROUPS of <=8 nodes whose total in-degree is <=128
(snake-deal over degree-sorted nodes + local repair). Every edge is routed to
its destination node's group; a group's edges (padded to 128) form one matmul
chunk. 16 groups = one WINDOW of 128 node slots; 50 windows per core.

Device side, per window: for each of the 16 chunks, a single [128e x 128f]^T @
[128e x 8v] one-hot matmul segment-sums the chunk's edges into its own
disjoint 8-column slice of the window's PSUM accumulator ([feat, node]
orientation, no accumulation needed). Then the MeshGraphMLP
(Linear->SiLU->Linear->LayerNorm) + residual runs per window on-chip, with
sqrt batched across windows to avoid ACT table thrash. Edge features and
one-hots travel in bf16 (exact 0/1 one-hots; fp32 PSUM accumulate); everything
downstream of the segment-sum is fp32.
"""
import os
import sys
if "/opt/trn_rl_repo" not in sys.path:
    sys.path.insert(0, "/opt/trn_rl_repo")

import numpy as np

N_NODES = 50000
D = 128
HID = 128
P = 128                      # SBUF partitions / edges per chunk / nodes per window
N_CORES = 8
CH = 16                      # chunks (groups) per window
GN = 8                       # node slots per group
GE = 128                     # edge capacity per group
BATCH = 10                   # windows per output DMA / sqrt batch
EFB = 4                      # windows per efeat DMA

_program_cache: dict = {}


# ----------------------------------------------------------------------------
# Host-side preprocessing
# ----------------------------------------------------------------------------

def _pack_groups(deg, n_groups):
    """Snake-deal degree-sorted nodes into groups of <=GN nodes / <=GE edges,
    then repair the few sum-cap violations by swapping with light groups.
    Returns (node_grp, node_rel) or None if infeasible."""
    n = len(deg)
    order = np.argsort(-deg, kind="stable")
    node_grp = np.full(n, -1, np.int32)
    for l in range(GN):
        lo, hi = l * n_groups, min((l + 1) * n_groups, n)
        if lo >= n:
            break
        idx = order[lo:hi]
        g = np.arange(hi - lo)
        if l % 2:
            g = n_groups - 1 - g
        node_grp[idx] = g
    gsum = np.bincount(node_grp, weights=deg, minlength=n_groups).astype(np.int64)
    members = [[] for _ in range(n_groups)]
    for node in order:
        members[node_grp[node]].append(node)

    over = list(np.where(gsum > GE)[0])
    if over:
        cand = np.argsort(gsum)[:4000].tolist()
        for g in over:
            guard = 0
            while gsum[g] > GE and guard < 200:
                guard += 1
                done = False
                for a in sorted(members[g], key=lambda x: -deg[x]):
                    for u in cand:
                        if u == g or gsum[u] > GE or not members[u]:
                            continue
                        b = min(members[u], key=lambda x: deg[x])
                        if deg[a] > deg[b] and gsum[u] - deg[b] + deg[a] <= GE:
                            members[g].remove(a)
                            members[u].remove(b)
                            members[g].append(b)
                            members[u].append(a)
                            node_grp[a], node_grp[b] = u, g
                            dd = int(deg[a] - deg[b])
                            gsum[g] -= dd
                            gsum[u] += dd
                            done = True
                            break
                    if done:
                        break
                if not done:
                    return None
    if gsum.max() > GE:
        return None
    node_rel = np.empty(n, np.int32)
    for g in range(n_groups):
        for i, node in enumerate(members[g]):
            node_rel[node] = i
    return node_grp, node_rel


def _preprocess(efeat, nfeat, dst_idx, ln_b):
    fp16 = np.dtype(np.float16)
    n_nodes = nfeat.shape[0]
    n_edges = efeat.shape[0]
    dst = np.asarray(dst_idx).astype(np.int64)
    deg = np.bincount(dst, minlength=n_nodes)
    if deg.max() > GE:
        raise ValueError(f"node degree {deg.max()} exceeds group capacity {GE}")

    for W in (50, 51, 52, 54, 58, 64):
        n_groups = N_CORES * W * CH
        if n_groups * GN < n_nodes or n_groups * GE < n_edges:
            continue
        r = _pack_groups(deg, n_groups)
        if r is not None:
            break
    else:
        raise ValueError("group packing failed")
    node_grp, node_rel = r
    W_TOT = N_CORES * W
    node_slots = W_TOT * P

    # Route each edge to (window, chunk, partition) of its destination group.
    g_of_edge = node_grp[dst]
    edge_perm = np.argsort(g_of_edge, kind="stable")
    gsorted = g_of_edge[edge_perm]
    counts = np.bincount(gsorted, minlength=n_groups)
    starts = np.concatenate([[0], np.cumsum(counts)[:-1]])
    j_within = np.arange(n_edges, dtype=np.int64) - np.repeat(starts, counts)
    w = gsorted.astype(np.int64) // CH
    c = gsorted.astype(np.int64) % CH
    p = j_within
    flat_row = (w * P + p) * CH + c

    efeat_dev = np.zeros((W_TOT * P * CH, D), fp16)
    efeat_dev[flat_row] = efeat[edge_perm].astype(fp16)
    rel_dev = np.zeros((W_TOT * P, CH), fp16)
    rel_dev[w * P + p, c] = node_rel[dst[edge_perm]].astype(fp16)

    nfeat_perm = np.zeros((node_slots, D), np.float32)
    slot_of_node = node_grp.astype(np.int64) * GN + node_rel
    nfeat_perm[slot_of_node] = nfeat
    nfb_perm = nfeat_perm + np.asarray(ln_b, np.float32)[None, :]

    return dict(efeat_dev=efeat_dev, rel_dev=rel_dev, nfeat_perm=nfeat_perm,
                nfb_perm=nfb_perm, slot_of_node=slot_of_node, W=W)


def _build_in_maps(pre, w1, b1, w2, b2, ln_g):
    fp16 = np.dtype(np.float16)
    W = pre["W"]
    W_TOT = N_CORES * W
    efeat_dev = pre["efeat_dev"].reshape(W_TOT, P, CH, D)
    rel_dev = pre["rel_dev"].reshape(W_TOT, P, CH)
    nfeat_perm = pre["nfeat_perm"]
    nfb_perm = pre["nfb_perm"]

    iota = np.ascontiguousarray(
        np.broadcast_to(np.arange(GN).astype(fp16), (P, CH, GN)))
    w1 = np.asarray(w1, np.float32)
    w1a = np.ascontiguousarray(w1[:D])
    w1b = np.ascontiguousarray(w1[D:].astype(fp16))
    w2c = np.ascontiguousarray(np.asarray(w2, np.float32))
    b1c = np.ascontiguousarray(np.asarray(b1, np.float32)[:, None])
    grep = np.ascontiguousarray(
        np.broadcast_to(np.asarray(ln_g, np.float32), (P, D)))
    b2rep = np.ascontiguousarray(
        np.broadcast_to(np.asarray(b2, np.float32), (P, D)))

    in_maps = []
    for cidx in range(N_CORES):
        sl = slice(cidx * W, (cidx + 1) * W)
        nsl = slice(cidx * W * P, (cidx + 1) * W * P)
        in_maps.append(dict(
            ef=np.ascontiguousarray(efeat_dev[sl]),
            rel=np.ascontiguousarray(rel_dev[sl].transpose(1, 0, 2)),
            iota=iota,
            nfT=np.ascontiguousarray(nfeat_perm[nsl].T.astype(fp16)),
            nfb=np.ascontiguousarray(
                nfb_perm[nsl].reshape(W, P, D).transpose(1, 0, 2)),
            w1a=w1a, w1b=w1b, w2=w2c, b1=b1c, grep=grep, b2rep=b2rep,
        ))
    return in_maps


# ----------------------------------------------------------------------------
# Device program
# ----------------------------------------------------------------------------

def _build_program(W):
    import concourse.bass as bass
    import concourse.tile as tile
    from concourse import bacc, mybir
    from contextlib import ExitStack

    f32 = mybir.dt.float32
    fp16 = mybir.dt.float16
    nc = bacc.Bacc("TRN2", target_bir_lowering=False, debug=False,
                   enable_asserts=True, num_devices=N_CORES)

    ef = nc.dram_tensor("ef", [W, P, CH, D], fp16, kind="ExternalInput").ap()
    rel = nc.dram_tensor("rel", [P, W, CH], fp16, kind="ExternalInput").ap()
    iota = nc.dram_tensor("iota", [P, CH, GN], fp16, kind="ExternalInput").ap()
    nfT = nc.dram_tensor("nfT", [P, W * P], fp16, kind="ExternalInput").ap()
    nfb = nc.dram_tensor("nfb", [P, W, D], f32, kind="ExternalInput").ap()
    w1a = nc.dram_tensor("w1a", [D, HID], f32, kind="ExternalInput").ap()
    w1b = nc.dram_tensor("w1b", [D, HID], fp16, kind="ExternalInput").ap()
    w2 = nc.dram_tensor("w2", [HID, D], f32, kind="ExternalInput").ap()
    b1 = nc.dram_tensor("b1", [HID, 1], f32, kind="ExternalInput").ap()
    grep = nc.dram_tensor("grep", [P, D], f32, kind="ExternalInput").ap()
    b2rep = nc.dram_tensor("b2rep", [P, D], f32, kind="ExternalInput").ap()
    out = nc.dram_tensor("out", [P, W * D], f32, kind="ExternalOutput").ap()

    with ExitStack() as ctx:
        tc = ctx.enter_context(tile.TileContext(nc))
        consts = ctx.enter_context(tc.tile_pool(name="consts", bufs=1))
        ef_pool = ctx.enter_context(tc.tile_pool(name="ef", bufs=3))
        oh_pool = ctx.enter_context(tc.tile_pool(name="oh", bufs=3))
        agg_pool = ctx.enter_context(tc.tile_pool(name="agg", bufs=2))
        h_pool = ctx.enter_context(tc.tile_pool(name="h", bufs=2))
        x_pool = ctx.enter_context(tc.tile_pool(name="x", bufs=2 * BATCH + 2))
        xn_pool = ctx.enter_context(tc.tile_pool(name="xn", bufs=3))
        out_pool = ctx.enter_context(tc.tile_pool(name="outp", bufs=2))
        mv_pool = ctx.enter_context(tc.tile_pool(name="mv", bufs=3))
        stat_pool = ctx.enter_context(tc.tile_pool(name="stat", bufs=6))
        agg_ps = ctx.enter_context(tc.tile_pool(name="agg_ps", bufs=2, space="PSUM"))
        h1_ps = ctx.enter_context(tc.tile_pool(name="h1_ps", bufs=2, space="PSUM"))
        o2_ps = ctx.enter_context(tc.tile_pool(name="o2_ps", bufs=2, space="PSUM"))

        # Load constants (SWDGE so the big HWDGE queues stay free)
        t_iota = consts.tile([P, CH, GN], fp16)
        nc.gpsimd.dma_start(out=t_iota[:], in_=iota[:])
        t_rel = consts.tile([P, W, CH], fp16)
        nc.gpsimd.dma_start(out=t_rel[:], in_=rel[:])
        t_nfT = consts.tile([P, W * P], fp16)
        nc.scalar.dma_start(out=t_nfT[:], in_=nfT[:])
        t_nfb = consts.tile([P, W, D], f32)
        nc.scalar.dma_start(out=t_nfb[:], in_=nfb[:])
        t_w1a = consts.tile([D, HID], f32)
        nc.gpsimd.dma_start(out=t_w1a[:], in_=w1a[:])
        t_w1b = consts.tile([D, HID], fp16)
        nc.gpsimd.dma_start(out=t_w1b[:], in_=w1b[:])
        t_w2 = consts.tile([HID, D], f32)
        nc.gpsimd.dma_start(out=t_w2[:], in_=w2[:])
        t_b1 = consts.tile([HID, 1], f32)
        nc.gpsimd.dma_start(out=t_b1[:], in_=b1[:])
        t_grep = consts.tile([P, D], f32)
        nc.gpsimd.dma_start(out=t_grep[:], in_=grep[:])
        t_b2rep = consts.tile([P, D], f32)
        nc.gpsimd.dma_start(out=t_b2rep[:], in_=b2rep[:])
        t_eps = consts.tile([P, 1], f32)
        nc.vector.memset(t_eps[:], 1e-5)

        AF = mybir.ActivationFunctionType
        OP = mybir.AluOpType

        xs = [None] * BATCH
        mv_b = None
        out_tile = None
        eft = None

        for w in range(W):
            b = w % BATCH
            blk = w // BATCH
            if b == 0:
                out_tile = out_pool.tile([P, BATCH * D], f32)
                mv_b = mv_pool.tile([P, BATCH, 2], f32, tag="mv")

            # efeat DMA: EFB windows (2MB fp16) at a time
            if w % EFB == 0:
                nw = min(EFB, W - w)
                eft = ef_pool.tile([P, EFB, CH, D], fp16, tag="eft")
                nc.sync.dma_start(out=eft[:, :nw],
                                  in_=ef[w:w + nw].rearrange("w p c d -> p w c d"))
            efw = eft[:, w % EFB]

            # one-hot: oh[p, c, v] = (rel[p, w, c] == v), v in [0, 8)
            oh = oh_pool.tile([P, CH, GN], fp16)
            nc.vector.tensor_tensor(
                out=oh[:],
                in0=t_rel[:, w, :, None].to_broadcast([P, CH, GN]),
                in1=t_iota[:],
                op=OP.is_equal,
            )

            # aggT[f, c*8+v] = efw[:, c, :].T @ oh[:, c, :]  (disjoint cols)
            aggp = agg_ps.tile([P, CH * GN], f32, space="PSUM")
            for c in range(CH):
                nc.tensor.matmul(
                    out=aggp[:, c * GN:(c + 1) * GN],
                    lhsT=efw[:, c, :],
                    rhs=oh[:, c, :],
                    start=True,
                    stop=True,
                )
            aggs = agg_pool.tile([P, P], f32)
            nc.any.tensor_copy(out=aggs[:], in_=aggp[:])

            # h1T[hid, v] = w1a.T @ aggT + w1b.T @ nfT_w ; h = silu(h1T + b1)
            h1p = h1_ps.tile([HID, P], f32, space="PSUM")
            nc.tensor.matmul(out=h1p[:], lhsT=t_w1a[:], rhs=aggs[:],
                             start=True, stop=False)
            nc.tensor.matmul(out=h1p[:], lhsT=t_w1b[:],
                             rhs=t_nfT[:, w * P:(w + 1) * P],
                             start=False, stop=True)
            h = h_pool.tile([HID, P], f32)
            nc.scalar.activation(out=h[:], in_=h1p[:], func=AF.Silu,
                                 bias=t_b1[:], scale=1.0)

            # o2[v, f] = h.T @ w2 ; x = o2 + b2
            o2p = o2_ps.tile([P, D], f32, space="PSUM")
            nc.tensor.matmul(out=o2p[:], lhsT=h[:], rhs=t_w2[:],
                             start=True, stop=True)
            x = x_pool.tile([P, D], f32, tag="x")
            nc.vector.tensor_tensor(out=x[:], in0=o2p[:], in1=t_b2rep[:],
                                    op=OP.add)

            # LayerNorm stats (normalize at batch end: one Sqrt per batch)
            stats = stat_pool.tile([P, 6], f32)
            nc.vector.bn_stats(out=stats[:], in_=x[:])
            nc.vector.bn_aggr(out=mv_b[:, b, :], in_=stats[:])
            xs[b] = x

            if b == BATCH - 1:
                sd_b = stat_pool.tile([P, BATCH], f32, tag="sd")
                nc.scalar.activation(out=sd_b[:], in_=mv_b[:, :, 1],
                                     func=AF.Sqrt, bias=t_eps[:], scale=1.0)
                rstd_b = stat_pool.tile([P, BATCH], f32, tag="rstd")
                nc.vector.reciprocal(out=rstd_b[:], in_=sd_b[:])

                for i in range(BATCH):
                    xn = xn_pool.tile([P, D], f32)
                    nc.vector.tensor_scalar(out=xn[:], in0=xs[i][:],
                                            scalar1=mv_b[:, i, 0:1],
                                            scalar2=rstd_b[:, i:i + 1],
                                            op0=OP.subtract, op1=OP.mult)
                    wg = blk * BATCH + i
                    xg = xn_pool.tile([P, D], f32, tag="xg")
                    nc.gpsimd.tensor_mul(out=xg[:], in0=xn[:], in1=t_grep[:])
                    nc.gpsimd.tensor_add(out=out_tile[:, i * D:(i + 1) * D],
                                         in0=xg[:], in1=t_nfb[:, wg, :])

                nc.scalar.dma_start(
                    out=out[:, blk * BATCH * D:(blk + 1) * BATCH * D],
                    in_=out_tile[:])

    nc.finalize()
    return nc


def _get_program(W):
    if W not in _program_cache:
        _program_cache[W] = _build_program(W)
    return _program_cache[W]


# ----------------------------------------------------------------------------
# Entry point
# ----------------------------------------------------------------------------

def kernel(efeat, nfeat, dst_idx, w1, b1, w2, b2, ln_g, ln_b):
    from concourse.bass_utils import run_bass_kernel_spmd

    efeat = np.asarray(efeat, np.float32)
    nfeat = np.asarray(nfeat, np.float32)
    pre = _preprocess(efeat, nfeat, dst_idx, ln_b)
    W = pre["W"]
    nc = _get_program(W)
    in_maps = _build_in_maps(pre, w1, b1, w2, b2, ln_g)

    res = run_bass_kernel_spmd(nc, in_maps, list(range(N_CORES)))

    node_slots = N_CORES * W * P
    out_slots = np.empty((node_slots, D), np.float32)
    for cidx in range(N_CORES):
        oc = res.results[cidx]["out"].reshape(P, W, D).transpose(1, 0, 2)
        out_slots[cidx * W * P:(cidx + 1) * W * P] = oc.reshape(W * P, D)
    return out_slots[pre["slot_of_node"]]


# revision 11
# speedup vs baseline: 384.8835x; 1.1506x over previous
"""Trainium2 Bass kernel for GNN NodeBlock (segment-sum + MLP + LayerNorm + residual).

Strategy: shard NODES across the 8 cores (no collectives needed).

Host side packs nodes into GROUPS of <=8 nodes whose total in-degree is <=128
(snake-deal over degree-sorted nodes + local repair). Every edge is routed to
its destination node's group; a group's edges (padded to 128) form one matmul
chunk. 16 groups = one WINDOW of 128 node slots; 50 windows per core.

Device side, per window: for each of the 16 chunks, a single [128e x 128f]^T @
[128e x 8v] one-hot matmul segment-sums the chunk's edges into its own
disjoint 8-column slice of the window's PSUM accumulator ([feat, node]
orientation, no accumulation needed). Then the MeshGraphMLP
(Linear->SiLU->Linear->LayerNorm) + residual runs per window on-chip, with
sqrt batched across windows to avoid ACT table thrash. Edge features and
one-hots travel in bf16 (exact 0/1 one-hots; fp32 PSUM accumulate); everything
downstream of the segment-sum is fp32.
"""
import os
import sys
if "/opt/trn_rl_repo" not in sys.path:
    sys.path.insert(0, "/opt/trn_rl_repo")

import numpy as np

N_NODES = 50000
D = 128
HID = 128
P = 128                      # SBUF partitions / edges per chunk / nodes per window
N_CORES = 8
CH = 16                      # chunks (groups) per window
GN = 8                       # node slots per group
GE = 128                     # edge capacity per group
BATCH = 10                   # windows per output DMA / sqrt batch
EFB = 4                      # windows per efeat DMA

_program_cache: dict = {}


# ----------------------------------------------------------------------------
# Host-side preprocessing
# ----------------------------------------------------------------------------

def _pack_groups(deg, n_groups):
    """Snake-deal degree-sorted nodes into groups of <=GN nodes / <=GE edges,
    then repair the few sum-cap violations by swapping with light groups.
    Returns (node_grp, node_rel) or None if infeasible."""
    n = len(deg)
    order = np.argsort(-deg, kind="stable")
    node_grp = np.full(n, -1, np.int32)
    for l in range(GN):
        lo, hi = l * n_groups, min((l + 1) * n_groups, n)
        if lo >= n:
            break
        idx = order[lo:hi]
        g = np.arange(hi - lo)
        if l % 2:
            g = n_groups - 1 - g
        node_grp[idx] = g
    gsum = np.bincount(node_grp, weights=deg, minlength=n_groups).astype(np.int64)
    members = [[] for _ in range(n_groups)]
    for node in order:
        members[node_grp[node]].append(node)

    over = list(np.where(gsum > GE)[0])
    if over:
        cand = np.argsort(gsum)[:4000].tolist()
        for g in over:
            guard = 0
            while gsum[g] > GE and guard < 200:
                guard += 1
                done = False
                for a in sorted(members[g], key=lambda x: -deg[x]):
                    for u in cand:
                        if u == g or gsum[u] > GE or not members[u]:
                            continue
                        b = min(members[u], key=lambda x: deg[x])
                        if deg[a] > deg[b] and gsum[u] - deg[b] + deg[a] <= GE:
                            members[g].remove(a)
                            members[u].remove(b)
                            members[g].append(b)
                            members[u].append(a)
                            node_grp[a], node_grp[b] = u, g
                            dd = int(deg[a] - deg[b])
                            gsum[g] -= dd
                            gsum[u] += dd
                            done = True
                            break
                    if done:
                        break
                if not done:
                    return None
    if gsum.max() > GE:
        return None
    node_rel = np.empty(n, np.int32)
    for g in range(n_groups):
        for i, node in enumerate(members[g]):
            node_rel[node] = i
    return node_grp, node_rel


def _preprocess(efeat, nfeat, dst_idx, ln_b):
    fp16 = np.dtype(np.float16)
    n_nodes = nfeat.shape[0]
    n_edges = efeat.shape[0]
    dst = np.asarray(dst_idx).astype(np.int64)
    deg = np.bincount(dst, minlength=n_nodes)
    if deg.max() > GE:
        raise ValueError(f"node degree {deg.max()} exceeds group capacity {GE}")

    for W in (50, 51, 52, 54, 58, 64):
        n_groups = N_CORES * W * CH
        if n_groups * GN < n_nodes or n_groups * GE < n_edges:
            continue
        r = _pack_groups(deg, n_groups)
        if r is not None:
            break
    else:
        raise ValueError("group packing failed")
    node_grp, node_rel = r
    W_TOT = N_CORES * W
    node_slots = W_TOT * P

    # Route each edge to (window, chunk, partition) of its destination group.
    g_of_edge = node_grp[dst]
    edge_perm = np.argsort(g_of_edge, kind="stable")
    gsorted = g_of_edge[edge_perm]
    counts = np.bincount(gsorted, minlength=n_groups)
    starts = np.concatenate([[0], np.cumsum(counts)[:-1]])
    j_within = np.arange(n_edges, dtype=np.int64) - np.repeat(starts, counts)
    w = gsorted.astype(np.int64) // CH
    c = gsorted.astype(np.int64) % CH
    p = j_within
    flat_row = (w * P + p) * CH + c

    efeat_dev = np.zeros((W_TOT * P * CH, D), fp16)
    efeat_dev[flat_row] = efeat[edge_perm].astype(fp16)
    rel_dev = np.zeros((W_TOT * P, CH), fp16)
    rel_dev[w * P + p, c] = node_rel[dst[edge_perm]].astype(fp16)

    nfeat_perm = np.zeros((node_slots, D), np.float32)
    slot_of_node = node_grp.astype(np.int64) * GN + node_rel
    nfeat_perm[slot_of_node] = nfeat

    return dict(efeat_dev=efeat_dev, rel_dev=rel_dev, nfeat_perm=nfeat_perm,
                slot_of_node=slot_of_node, W=W)


def _build_in_maps(pre, w1, b1, w2, b2, ln_g, ln_b):
    fp16 = np.dtype(np.float16)
    W = pre["W"]
    W_TOT = N_CORES * W
    efeat_dev = pre["efeat_dev"].reshape(W_TOT, P, CH, D)
    rel_dev = pre["rel_dev"].reshape(W_TOT, P, CH)
    nfeat_perm = pre["nfeat_perm"]

    iota = np.ascontiguousarray(
        np.broadcast_to(np.arange(GN).astype(fp16), (P, CH, GN)))
    w1 = np.asarray(w1, np.float32)
    w1a = np.ascontiguousarray(w1[:D])
    w1b = np.ascontiguousarray(w1[D:].astype(fp16))
    w2c = np.ascontiguousarray(np.asarray(w2, np.float32))
    b1c = np.ascontiguousarray(np.asarray(b1, np.float32)[:, None])
    grep = np.ascontiguousarray(
        np.broadcast_to(np.asarray(ln_g, np.float32), (P, D)))
    b2rep = np.ascontiguousarray(
        np.broadcast_to(np.asarray(b2, np.float32), (P, D)))
    lnb = np.ascontiguousarray(np.asarray(ln_b, np.float32)[None, :].astype(fp16))
    id128 = np.ascontiguousarray(np.eye(P, dtype=fp16))

    in_maps = []
    for cidx in range(N_CORES):
        sl = slice(cidx * W, (cidx + 1) * W)
        nsl = slice(cidx * W * P, (cidx + 1) * W * P)
        in_maps.append(dict(
            ef=np.ascontiguousarray(efeat_dev[sl]),
            rel=np.ascontiguousarray(rel_dev[sl].transpose(1, 0, 2)),
            iota=iota,
            nfT=np.ascontiguousarray(nfeat_perm[nsl].T.astype(fp16)),
            w1a=w1a, w1b=w1b, w2=w2c, b1=b1c, grep=grep, b2rep=b2rep,
            lnb=lnb, id128=id128,
        ))
    return in_maps


# ----------------------------------------------------------------------------
# Device program
# ----------------------------------------------------------------------------

def _build_program(W):
    import concourse.bass as bass
    import concourse.tile as tile
    from concourse import bacc, mybir
    from contextlib import ExitStack

    f32 = mybir.dt.float32
    fp16 = mybir.dt.float16
    nc = bacc.Bacc("TRN2", target_bir_lowering=False, debug=False,
                   enable_asserts=True, num_devices=N_CORES)

    ef = nc.dram_tensor("ef", [W, P, CH, D], fp16, kind="ExternalInput").ap()
    rel = nc.dram_tensor("rel", [P, W, CH], fp16, kind="ExternalInput").ap()
    iota = nc.dram_tensor("iota", [P, CH, GN], fp16, kind="ExternalInput").ap()
    nfT = nc.dram_tensor("nfT", [P, W * P], fp16, kind="ExternalInput").ap()
    lnb = nc.dram_tensor("lnb", [1, D], fp16, kind="ExternalInput").ap()
    id128 = nc.dram_tensor("id128", [P, P], fp16, kind="ExternalInput").ap()
    w1a = nc.dram_tensor("w1a", [D, HID], f32, kind="ExternalInput").ap()
    w1b = nc.dram_tensor("w1b", [D, HID], fp16, kind="ExternalInput").ap()
    w2 = nc.dram_tensor("w2", [HID, D], f32, kind="ExternalInput").ap()
    b1 = nc.dram_tensor("b1", [HID, 1], f32, kind="ExternalInput").ap()
    grep = nc.dram_tensor("grep", [P, D], f32, kind="ExternalInput").ap()
    b2rep = nc.dram_tensor("b2rep", [P, D], f32, kind="ExternalInput").ap()
    out = nc.dram_tensor("out", [P, W * D], f32, kind="ExternalOutput").ap()

    with ExitStack() as ctx:
        tc = ctx.enter_context(tile.TileContext(nc))
        consts = ctx.enter_context(tc.tile_pool(name="consts", bufs=1))
        ef_pool = ctx.enter_context(tc.tile_pool(name="ef", bufs=3))
        oh_pool = ctx.enter_context(tc.tile_pool(name="oh", bufs=3))
        agg_pool = ctx.enter_context(tc.tile_pool(name="agg", bufs=2))
        h_pool = ctx.enter_context(tc.tile_pool(name="h", bufs=2))
        x_pool = ctx.enter_context(tc.tile_pool(name="x", bufs=2 * BATCH + 2))
        xn_pool = ctx.enter_context(tc.tile_pool(name="xn", bufs=3))
        out_pool = ctx.enter_context(tc.tile_pool(name="outp", bufs=2))
        mv_pool = ctx.enter_context(tc.tile_pool(name="mv", bufs=3))
        stat_pool = ctx.enter_context(tc.tile_pool(name="stat", bufs=6))
        agg_ps = ctx.enter_context(tc.tile_pool(name="agg_ps", bufs=2, space="PSUM"))
        h1_ps = ctx.enter_context(tc.tile_pool(name="h1_ps", bufs=2, space="PSUM"))
        o2_ps = ctx.enter_context(tc.tile_pool(name="o2_ps", bufs=2, space="PSUM"))
        nf_ps = ctx.enter_context(tc.tile_pool(name="nf_ps", bufs=2, space="PSUM"))

        # Load constants (SWDGE so the big HWDGE queues stay free)
        t_iota = consts.tile([P, CH, GN], fp16)
        nc.gpsimd.dma_start(out=t_iota[:], in_=iota[:])
        t_rel = consts.tile([P, W, CH], fp16)
        nc.gpsimd.dma_start(out=t_rel[:], in_=rel[:])
        t_nfT = consts.tile([P, W * P], fp16)
        nc.scalar.dma_start(out=t_nfT[:], in_=nfT[:])
        t_lnb = consts.tile([1, D], fp16)
        nc.gpsimd.dma_start(out=t_lnb[:], in_=lnb[:])
        t_id = consts.tile([P, P], fp16)
        nc.gpsimd.dma_start(out=t_id[:], in_=id128[:])
        t_ones = consts.tile([1, P], fp16)
        nc.vector.memset(t_ones[:], 1.0)
        t_w1a = consts.tile([D, HID], f32)
        nc.gpsimd.dma_start(out=t_w1a[:], in_=w1a[:])
        t_w1b = consts.tile([D, HID], fp16)
        nc.gpsimd.dma_start(out=t_w1b[:], in_=w1b[:])
        t_w2 = consts.tile([HID, D], f32)
        nc.gpsimd.dma_start(out=t_w2[:], in_=w2[:])
        t_b1 = consts.tile([HID, 1], f32)
        nc.gpsimd.dma_start(out=t_b1[:], in_=b1[:])
        t_grep = consts.tile([P, D], f32)
        nc.gpsimd.dma_start(out=t_grep[:], in_=grep[:])
        t_b2rep = consts.tile([P, D], f32)
        nc.gpsimd.dma_start(out=t_b2rep[:], in_=b2rep[:])
        t_eps = consts.tile([P, 1], f32)
        nc.vector.memset(t_eps[:], 1e-5)

        AF = mybir.ActivationFunctionType
        OP = mybir.AluOpType

        xs = [None] * BATCH
        mv_b = None
        out_tile = None
        eft = None

        for w in range(W):
            b = w % BATCH
            blk = w // BATCH
            if b == 0:
                out_tile = out_pool.tile([P, BATCH * D], f32)
                mv_b = mv_pool.tile([P, BATCH, 2], f32, tag="mv")

            # efeat DMA: EFB windows (2MB fp16) at a time
            if w % EFB == 0:
                nw = min(EFB, W - w)
                eft = ef_pool.tile([P, EFB, CH, D], fp16, tag="eft")
                nc.sync.dma_start(out=eft[:, :nw],
                                  in_=ef[w:w + nw].rearrange("w p c d -> p w c d"))
            efw = eft[:, w % EFB]

            # one-hot: oh[p, c, v] = (rel[p, w, c] == v), v in [0, 8)
            oh = oh_pool.tile([P, CH, GN], fp16)
            nc.vector.tensor_tensor(
                out=oh[:],
                in0=t_rel[:, w, :, None].to_broadcast([P, CH, GN]),
                in1=t_iota[:],
                op=OP.is_equal,
            )

            # aggT[f, c*8+v] = efw[:, c, :].T @ oh[:, c, :]  (disjoint cols)
            aggp = agg_ps.tile([P, CH * GN], f32, space="PSUM")
            for c in range(CH):
                nc.tensor.matmul(
                    out=aggp[:, c * GN:(c + 1) * GN],
                    lhsT=efw[:, c, :],
                    rhs=oh[:, c, :],
                    start=True,
                    stop=True,
                )
            aggs = agg_pool.tile([P, P], f32)
            nc.any.tensor_copy(out=aggs[:], in_=aggp[:])

            # h1T[hid, v] = w1a.T @ aggT + w1b.T @ nfT_w ; h = silu(h1T + b1)
            h1p = h1_ps.tile([HID, P], f32, space="PSUM")
            nc.tensor.matmul(out=h1p[:], lhsT=t_w1a[:], rhs=aggs[:],
                             start=True, stop=False)
            nc.tensor.matmul(out=h1p[:], lhsT=t_w1b[:],
                             rhs=t_nfT[:, w * P:(w + 1) * P],
                             start=False, stop=True)
            h = h_pool.tile([HID, P], f32)
            nc.scalar.activation(out=h[:], in_=h1p[:], func=AF.Silu,
                                 bias=t_b1[:], scale=1.0)

            # o2[v, f] = h.T @ w2 ; x = o2 + b2
            o2p = o2_ps.tile([P, D], f32, space="PSUM")
            nc.tensor.matmul(out=o2p[:], lhsT=h[:], rhs=t_w2[:],
                             start=True, stop=True)
            x = x_pool.tile([P, D], f32, tag="x")
            nc.vector.tensor_tensor(out=x[:], in0=o2p[:], in1=t_b2rep[:],
                                    op=OP.add)

            # LayerNorm stats (normalize at batch end: one Sqrt per batch)
            stats = stat_pool.tile([P, 6], f32)
            nc.vector.bn_stats(out=stats[:], in_=x[:])
            nc.vector.bn_aggr(out=mv_b[:, b, :], in_=stats[:])
            xs[b] = x

            if b == BATCH - 1:
                sd_b = stat_pool.tile([P, BATCH], f32, tag="sd")
                nc.scalar.activation(out=sd_b[:], in_=mv_b[:, :, 1],
                                     func=AF.Sqrt, bias=t_eps[:], scale=1.0)
                rstd_b = stat_pool.tile([P, BATCH], f32, tag="rstd")
                nc.vector.reciprocal(out=rstd_b[:], in_=sd_b[:])

                for i in range(BATCH):
                    wg = blk * BATCH + i
                    xn = xn_pool.tile([P, D], f32)
                    nc.vector.tensor_scalar(out=xn[:], in0=xs[i][:],
                                            scalar1=mv_b[:, i, 0:1],
                                            scalar2=rstd_b[:, i:i + 1],
                                            op0=OP.subtract, op1=OP.mult)
                    # nfbp[v, f] = nfeat[v, f] + ln_b (PE transpose + rank-1)
                    nfbp = nf_ps.tile([P, D], f32, space="PSUM")
                    nc.tensor.matmul(out=nfbp[:], lhsT=t_ones[:], rhs=t_lnb[:],
                                     start=True, stop=False)
                    nc.tensor.matmul(out=nfbp[:],
                                     lhsT=t_nfT[:, wg * P:(wg + 1) * P],
                                     rhs=t_id[:], start=False, stop=True)
                    xg = xn_pool.tile([P, D], f32, tag="xg")
                    nc.any.tensor_mul(out=xg[:], in0=xn[:], in1=t_grep[:])
                    nc.any.tensor_add(out=out_tile[:, i * D:(i + 1) * D],
                                      in0=xg[:], in1=nfbp[:])

                nc.scalar.dma_start(
                    out=out[:, blk * BATCH * D:(blk + 1) * BATCH * D],
                    in_=out_tile[:])

    nc.finalize()
    return nc


def _get_program(W):
    if W not in _program_cache:
        _program_cache[W] = _build_program(W)
    return _program_cache[W]


# ----------------------------------------------------------------------------
# Entry point
# ----------------------------------------------------------------------------

def kernel(efeat, nfeat, dst_idx, w1, b1, w2, b2, ln_g, ln_b):
    from concourse.bass_utils import run_bass_kernel_spmd

    efeat = np.asarray(efeat, np.float32)
    nfeat = np.asarray(nfeat, np.float32)
    pre = _preprocess(efeat, nfeat, dst_idx, ln_b)
    W = pre["W"]
    nc = _get_program(W)
    in_maps = _build_in_maps(pre, w1, b1, w2, b2, ln_g, ln_b)

    res = run_bass_kernel_spmd(nc, in_maps, list(range(N_CORES)))

    node_slots = N_CORES * W * P
    out_slots = np.empty((node_slots, D), np.float32)
    for cidx in range(N_CORES):
        oc = res.results[cidx]["out"].reshape(P, W, D).transpose(1, 0, 2)
        out_slots[cidx * W * P:(cidx + 1) * W * P] = oc.reshape(W * P, D)
    return out_slots[pre["slot_of_node"]]
